# revision 1
# baseline (speedup 1.0000x reference)
"""Trainium2 Bass kernel for per-head bilinear graph attention.

Reference computation (B=4, N=2048, IN=256, H=8, ATN=32):
    xt     = einsum('bni,hio->bhno', x, W) + b          # [B,H,N,32]
    xC     = einsum('bhno,hpo->bhnp', xt, C)            # [B,H,N,32]
    scores = einsum('bhnp,bhmp->bhnm', xC, xt)          # [B,H,N,N]
    alpha  = tanh(scores * adj[:,None])                 # [B,H,N,N]
    heads  = einsum('bhnm,bhmo->bhno', alpha, xt)       # [B,H,N,32]
    out    = concat heads on feature dim                # [B,N,256]

Sharding: 8 cores = 4 batches x 2 head-groups (4 heads each). Fully
data-parallel, no collectives. Each core computes out[b, :, hg*128:(hg+1)*128]
transposed ([128, 2048]); the host transposes back and concatenates.

Device-side layout is fully transposed ("T" = [feature/m, n]):
    xtT  [128(4h x 32o), 2048n]   stacked per-head xt^T (bias included)
    xCT  [128(4h x 32p), 2048n]   stacked per-head xC^T
    sT   [128m, n]     = scores[n, m]   (psum, per m-chunk per head)
    z    = sT * adjT   (adjT host-pretransposed so it is [m, n])
    alphaT = tanh(z)
    outT [128(4h x 32o), 2048n] accumulated in psum over 16 m-chunks

Engine budget per core: the kernel sits at the DVE+ACT joint capacity
floor. Mask-multiply columns (131072 x 1024-col units total) can only run
on DVE (1.18us/unit from fp32 psum) or, after an ACT psum->bf16 cast
(1.05us), on GPSIMD (2.2us, but off the critical engines). tanh columns
are ACT-only (0.93us/unit). Every psum->sbuf byte must pass through
DVE or ACT (GPSIMD and DMA have no PSUM port), so the equilibrium is
ACT ~= DVE ~= 145-150us busy; measured exec 175-215us depending on a
~25% run-to-run clock lottery (compare runs via exec/TT-duration).
Design choices vs the v1 baseline:
 - 14 multiply slices (hp0,q0 of even m-chunks, pairs 0-6) are cast by ACT
   and multiplied on GPSIMD: each costs ACT 1.05us but saves a FULL DVE
   column (the old cast->DVE-2x trick saved only half a column).
 - pair tanh in 2 x 4096-col instructions, odd half emitted first; the
   even half (holding the pool slice) is ready BEFORE the odd half, so
   the (reordering!) Tile scheduler cannot stall ACT on the Pool latency.
 - copies split for balance: xCT 0/1 + bias on ACT (prologue-latency),
   xCT 2/3 + xt4 + outT on DVE; out stored bf16 (host upcasts).
 - prologue: only proj/xC chunks 0,1 gate the start; chunks 2,3 and the
   xt4 transpose groups are spread one-per-unit through the first 8 units
   (PROLOG_AT). bias/xT0/W DMAs issued first on the SP queue (DMA issue
   serializes ~0.6us each; completion semaphores add ~2.6us).
 - deep buffer rotations (adj 6, z-pairs 4, alpha 3) because slot-reuse
   WAR deps otherwise propagate Pool/tanh lateness into DVE stalls.
 - po split into two 1-bank accumulators so q0's output copy overlaps
   q1's last matmuls.

Key facts baked into this design (from HW traces):
 - fp32 matmuls on TRN2 lower to TWO hw passes; bf16 operands halve PE time.
 - DVE tensor_tensor from PSUM runs at 1x (fp32); GPSIMD cannot access PSUM
   at all, so its multiplies need an ACT cast psum->sbuf first.
 - K=32 scores matmuls are packed pairwise into PE row-groups (tile_position)
   with [128,2,512] psum tiles; outT uses 4-way col-group packing.
 - PSUM budget: 3x2-bank scores slots + 1x2-bank output accumulator = 8.
 - Accumulating matmul groups in shared banks are seeded by a K=1 zeroing
   matmul so every real matmul uses start=False (safe under per-partition
   OR bank-wide has_written-clear semantics).
"""

import sys
import types

import numpy as np
import ml_dtypes

BF16_NP = ml_dtypes.bfloat16


def _ensure_axon_ntff_hook():
    """Provide antenv.axon_hooks if the image lacks it, so
    run_bass_kernel_spmd(trace=True) can capture NTFF profiles instead of
    crashing on the import. No-op when the real module exists."""
    try:
        import antenv.axon_hooks  # noqa: F401

        return
    except ImportError:
        pass
    mod = types.ModuleType("antenv.axon_hooks")
    _state = {"hook": None}
    mod.set_axon_ntff_profile_hook = lambda h: _state.__setitem__("hook", h)
    mod.get_axon_ntff_profile_hook = lambda: _state["hook"]
    sys.modules["antenv.axon_hooks"] = mod
    try:
        import antenv

        antenv.axon_hooks = mod
    except ImportError:
        pass
    try:
        from trn_agent_boot.trn_boot import _ntff_profile_via_ctypes

        mod.set_axon_ntff_profile_hook(
            _ntff_profile_via_ctypes("/opt/axon/libaxon_pjrt.so")
        )
    except Exception:
        pass


_ensure_axon_ntff_hook()

from concourse import bacc, mybir, tile
import concourse.bass as bass
from concourse.bass_utils import run_bass_kernel_spmd
from concourse.bass import _add_dep_helper

F32 = mybir.dt.float32
BF16 = mybir.dt.bfloat16
AF = mybir.ActivationFunctionType
ALU = mybir.AluOpType

P = 128
B, N, IN_DIM, H, ATN = 4, 2048, 256, 8, 32
NH = 4                # heads per core
NCORES = 8
MC = N // P           # 16 m-chunks
IC = IN_DIM // P      # 2 contraction chunks for the input projection

# Multiply slices routed ACT-cast -> GPSIMD: the (hp,0) slices of EVEN
# m-chunks, pairs 0..6. Their tanh half is emitted SECOND within the pair
# (after the odd chunk's 4096-col half), so the Pool multiply has a full
# tanh instruction (~5us) of latency margin with no schedule gymnastics.
POOL_PAIRS = (0, 1, 2, 3, 4, 5, 6)
POOL_HPS = (0,)  # which head-pairs of (even mc, q=0) go to the pool

_CACHE = {}


def build_graph():
    nc = bacc.Bacc("TRN2", target_bir_lowering=False, debug=False)

    xT_d = nc.dram_tensor("xT", [IN_DIM, N], BF16, kind="ExternalInput")
    id_d = nc.dram_tensor("ident", [P, P], BF16, kind="ExternalInput")
    adjT_d = nc.dram_tensor("adjT", [N, N], BF16, kind="ExternalInput")
    # weights: [P, IC*NH*ATN] W-part ++ [P, ATN] C^T-part, one fast DMA
    W_d = nc.dram_tensor("Wt", [P, IC * NH * ATN + ATN], BF16, kind="ExternalInput")
    b_d = nc.dram_tensor("bias", [P, 1], F32, kind="ExternalInput")
    out_d = nc.dram_tensor("out", [P, N], BF16, kind="ExternalOutput")

    with tile.TileContext(nc) as tc:
        with (
            tc.tile_pool(name="const", bufs=1) as cp,
            tc.tile_pool(name="adj", bufs=6) as adjp,
            tc.tile_pool(name="z", bufs=4) as zp,
            tc.tile_pool(name="alpha", bufs=3) as alp,
            tc.tile_pool(name="cast", bufs=4) as scp,
            tc.tile_pool(name="ps_o", bufs=2, space="PSUM") as ps_o,
            tc.tile_pool(name="ps_s", bufs=3, space="PSUM") as ps_s,
        ):
            # The latency-critical loads (bias, xT chunk0, weights) go on the
            # idle GPSIMD queue FIRST: DMA issues serialize at ~0.6us each
            # per sequencer and completion semaphores add ~2.6us, so issue
            # order directly sets when the projection chain can start.
            b_sb = cp.tile([P, 1], F32)
            nc.sync.dma_start(b_sb[:], b_d[:])
            xT_sb = cp.tile([P, IC, N], BF16)
            xT_src = xT_d[:].rearrange("(c p) n -> p c n", p=P)
            xt_dmas = [
                nc.sync.dma_start(
                    xT_sb[:, :, bass.ts(0, 512)], xT_src[:, :, bass.ts(0, 512)]
                )
            ]
            Wall_sb = cp.tile([P, IC * NH * ATN + ATN], BF16)
            nc.sync.dma_start(Wall_sb[:], W_d[:])
            for nq in range(1, N // 512):
                xt_dmas.append(
                    nc.gpsimd.dma_start(
                        xT_sb[:, :, bass.ts(nq, 512)],
                        xT_src[:, :, bass.ts(nq, 512)],
                    )
                )
            ident = cp.tile([P, P], BF16)
            nc.gpsimd.dma_start(ident[:], id_d[:])
            W_sb = Wall_sb[:, : IC * NH * ATN].rearrange(
                "p (c h o) -> p c h o", c=IC, h=NH
            )
            CT_sb = Wall_sb[:, IC * NH * ATN :]

            xtT = cp.tile([P, N], BF16)
            xCT = cp.tile([P, N], BF16)
            xt4 = cp.tile([P, MC, P], BF16)
            out_sb = cp.tile([P, N], BF16)
            zrow = cp.tile([1, 512], BF16)
            nc.gpsimd.memset(zrow[:], 0.0)

            # --- prologue pieces ---
            def emit_xtT(nq):
                # xtT[32h+o, n] = sum_i W[h,i,o] x[n,i] + b[h,o].
                # Zero-seed the bank, then accumulate with start=False
                # everywhere (model-independent safety); c-outer/h-inner so
                # the 4 col-groups run concurrently in the PE array. The
                # bias rides on the ACT copy out of psum.
                pt = ps_s.tile([P, 1024], F32, tag="s", name=f"pj_{nq}")
                nc.tensor.matmul(
                    pt[:, :512],
                    zrow[:, :P],
                    zrow[:, :512],
                    start=True,
                    stop=False,
                    skip_group_check=True,
                )
                for c in range(IC):
                    for h in range(NH):
                        nc.tensor.matmul(
                            pt[bass.ts(h, ATN), :512],
                            W_sb[:, c, h, :],
                            xT_sb[:, c, bass.ts(nq, 512)],
                            start=False,
                            stop=(c == IC - 1 and h == NH - 1),
                            tile_position=(0, h * ATN),
                            skip_group_check=True,
                        )
                nc.scalar.activation(
                    xtT[:, bass.ts(nq, 512)], pt[:, :512], AF.Identity, bias=b_sb[:]
                )

            def emit_xCT(nq, on_act=True):
                # xCT[32h+p, n] = sum_o C[h,p,o] xt[n,o]; diagonal 32x32
                # tiles run concurrently in distinct row+col groups.
                pt = ps_s.tile([P, 1024], F32, tag="s", name=f"xc_{nq}")
                for h in range(NH):
                    nc.tensor.matmul(
                        pt[bass.ts(h, ATN), :512],
                        CT_sb[bass.ts(h, ATN), :],
                        xtT[bass.ts(h, ATN), bass.ts(nq, 512)],
                        start=True,
                        stop=True,
                        tile_position=(h * ATN, h * ATN),
                        skip_group_check=True,
                    )
                if on_act:
                    nc.scalar.copy(xCT[:, bass.ts(nq, 512)], pt[:, :512])
                else:
                    nc.vector.tensor_copy(xCT[:, bass.ts(nq, 512)], pt[:, :512])

            def emit_xt4(g):
                # xt4[m_local, mc, f] = xt[mc*128+m_local, f]: PE transposes
                # of xtT, 4 m-chunks per psum tile (cycled through a ps_s
                # slot). Copies on DVE (bf16 psum -> bf16 sbuf, 2x path).
                pt = ps_s.tile([P, 4, P], BF16, tag="s", name=f"tr_{g}")
                for k in range(4):
                    nc.tensor.transpose(
                        pt[:, k, :], xtT[:, bass.ts(4 * g + k, P)], ident[:]
                    )
                nc.vector.tensor_copy(xt4[:, bass.ds(4 * g, 4), :], pt[:])

            # Only projection chunks 0,1 gate the first n-half (xCT cols
            # 0..1023); chunks 2,3 and the later transpose groups are
            # emitted inside the main loop where PE has slack, cutting the
            # serial prologue by ~8us.
            for nq in range(2):
                emit_xtT(nq)
                emit_xCT(nq)
            # (mc-chunk deps: unit mc needs xtT chunk mc//4; transpose group
            # g needs chunk g; heads(mc) needs xt4 group mc//4 by its pair's
            # tanh.) At most one extra PE task per unit so PE never falls a
            # whole task behind the DVE consumption rate.
            PROLOG_AT = {0: [lambda: emit_xt4(0)],
                         2: [lambda: emit_xtT(2)],
                         3: [lambda: emit_xCT(2, on_act=False)],
                         4: [lambda: emit_xt4(1)],
                         5: [lambda: emit_xtT(3)],
                         6: [lambda: emit_xCT(3, on_act=False)],
                         7: [lambda: emit_xt4(2)],
                         8: [lambda: emit_xt4(3)]}

            # --- main loop: n-half outer, m-chunks inner ---
            # Per (nh, mc): scores come out of PE in head-PAIR psum tiles
            # [128, 2, 512] so the two heads' K=32 matmuls run concurrently
            # in different PE row-groups while DVE still gets an FD=1024
            # multiply per instruction (adj broadcast over the pair dim).
            NHALF = N // 1024
            for nh in range(NHALF):
                # Two independent 1-bank accumulators so the q0 output copy
                # fires as soon as q0's last matmul stops, overlapping q1's.
                # Each is seeded by an explicit zeroing matmul (K=1, zero
                # weights) so every real outT matmul can use start=False:
                # correct regardless of whether the HW first-matmul
                # has_written clear is per-partition-slice or bank-wide.
                po_q = []
                for q in range(2):
                    pq = ps_o.tile([P, 512], F32, tag="po", name=f"po_{nh}_{q}")
                    po_q.append(pq)
                    nc.tensor.matmul(
                        pq[:],
                        zrow[:, :P],
                        zrow[:, :512],
                        start=True,
                        stop=False,
                        skip_group_check=True,
                    )

                zcur = [None]

                def emit_heads(mc, alpha, nh=nh):
                    for q in range(2):
                        for h in range(NH):
                            nc.tensor.matmul(
                                po_q[q][bass.ts(h, ATN), :],
                                xt4[:, mc, bass.ts(h, ATN)],
                                alpha[:, mc % 2, h, bass.ts(q, 512)],
                                start=False,
                                stop=(mc == MC - 1 and h == NH - 1),
                                tile_position=(0, h * ATN),
                                skip_group_check=True,
                            )

                for mc in range(MC):
                    p = mc // 2
                    if mc % 2 == 0:
                        zcur[0] = zp.tile(
                            [P, 2, NH, 1024], BF16, tag="z", name=f"zb_{nh}_{p}"
                        )
                    zb = zcur[0]
                    adjt = adjp.tile([P, 1024], BF16, tag="adj")
                    # no deferral: the critical loads already lead the sync
                    # queue (bias, xT0, W first; xT1-3 on gpsimd), and a
                    # deferred adj0 semaphore was measured gating the first
                    # multiply by 2.6us
                    nc.sync.dma_start(
                        adjt[:], adjT_d[bass.ts(mc, P), bass.ds(nh * 1024, 1024)]
                    )
                    pool_unit = (mc % 2 == 0) and p in POOL_PAIRS
                    for hp in range(NH // 2):
                        for q in range(2):
                            s2 = ps_s.tile([P, 2, 512], F32, tag="s")
                            for j in range(2):
                                h = 2 * hp + j
                                nc.tensor.matmul(
                                    s2[:, j, :],
                                    xtT[bass.ts(h, ATN), bass.ts(mc, P)],
                                    xCT[
                                        bass.ts(h, ATN),
                                        bass.ds(nh * 1024 + q * 512, 512),
                                    ],
                                    start=True,
                                    stop=True,
                                    tile_position=(h * ATN, 0),
                                    skip_group_check=True,
                                )
                            zsl = zb[:, mc % 2, bass.ds(2 * hp, 2), bass.ts(q, 512)]
                            adj_b = adjt[:, None, bass.ts(q, 512)].to_broadcast(
                                (P, 2, 512)
                            )
                            if pool_unit and q == 0 and hp in POOL_HPS:
                                # ACT casts psum->bf16; the otherwise-idle
                                # GPSIMD engine does the multiply (it has no
                                # PSUM port, hence the cast).
                                sc = scp.tile([P, 2, 512], BF16, tag="cast")
                                nc.scalar.copy(sc[:], s2[:])
                                nc.gpsimd.tensor_tensor(zsl, sc[:], adj_b, ALU.mult)
                            else:
                                nc.vector.tensor_tensor(zsl, s2[:], adj_b, ALU.mult)
                    if nh == 0:
                        for fn in PROLOG_AT.get(mc, ()):
                            fn()
                    if mc % 2 == 1 and mc < 15:
                        # pair tanh in two 4096-col instructions (measured
                        # optimum: one 8192-col instr saves ~370ns/instr of
                        # ACT access latency but costs ~10us in pipeline
                        # serialization; the 3-way split costs ~4.6us of
                        # extra overhead). The even half, which contains the
                        # pool-written slice, is ready BEFORE the odd half,
                        # so the reordering scheduler never stalls on Pool.
                        alpha = alp.tile(
                            [P, 2, NH, 1024], BF16, tag="alpha", name=f"al_{nh}_{p}"
                        )
                        nc.scalar.activation(alpha[:, 1], zb[:, 1], AF.Tanh)
                        emit_heads(mc, alpha)
                        nc.scalar.activation(alpha[:, 0], zb[:, 0], AF.Tanh)
                        emit_heads(mc - 1, alpha)
                    elif mc == 14:
                        # last pair fine-grained for tail latency; mc14 is
                        # not pool-routed so its tanh can fire immediately
                        alpha = alp.tile(
                            [P, 2, NH, 1024], BF16, tag="alpha", name=f"al_{nh}_{p}"
                        )
                        apair7 = alpha
                        nc.scalar.activation(alpha[:, 0], zb[:, 0], AF.Tanh)
                        emit_heads(14, alpha)
                    elif mc == 15:
                        alpha = apair7
                        for q in range(2):
                            nc.scalar.activation(
                                alpha[:, 1, :, bass.ts(q, 512)],
                                zb[:, 1, :, bass.ts(q, 512)],
                                AF.Tanh,
                            )
                            for h in range(NH):
                                nc.tensor.matmul(
                                    po_q[q][bass.ts(h, ATN), :],
                                    xt4[:, 15, bass.ts(h, ATN)],
                                    alpha[:, 1, h, bass.ts(q, 512)],
                                    start=False,
                                    stop=(h == NH - 1),
                                    tile_position=(0, h * ATN),
                                    skip_group_check=True,
                                )

                for q in range(2):
                    nc.vector.tensor_copy(
                        out_sb[:, bass.ds(nh * 1024 + q * 512, 512)],
                        po_q[q][:],
                    )
                    nc.sync.dma_start(
                        out_d[:, bass.ds(nh * 1024 + q * 512, 512)],
                        out_sb[:, bass.ds(nh * 1024 + q * 512, 512)],
                    )

    nc.compile()
    return nc


def _get_graph():
    if "nc" not in _CACHE:
        _CACHE["nc"] = build_graph()
    return _CACHE["nc"]


def make_in_maps(x, adj, W, b, C):
    in_maps = []
    for core in range(NCORES):
        bb = core // 2
        hg = core % 2
        hs = slice(hg * NH, (hg + 1) * NH)
        Wt = (
            W[hs]
            .reshape(NH, IC, P, ATN)
            .transpose(2, 1, 0, 3)
            .reshape(P, IC * NH * ATN)
        )
        CTt = C[hs].transpose(0, 2, 1).reshape(NH * ATN, ATN)
        in_maps.append(
            {
                "xT": np.ascontiguousarray(x[bb].T).astype(BF16_NP),
                "ident": np.eye(P, dtype=np.float32).astype(BF16_NP),
                "adjT": np.ascontiguousarray(adj[bb].T).astype(BF16_NP),
                "Wt": np.ascontiguousarray(
                    np.concatenate([Wt, CTt], axis=1)
                ).astype(BF16_NP),
                "bias": np.ascontiguousarray(b[hs].reshape(P, 1)),
            }
        )
    return in_maps


LAST_RESULT = None


def kernel(x, adj, W, b, C):
    global LAST_RESULT
    x = np.asarray(x, dtype=np.float32)
    adj = np.asarray(adj, dtype=np.float32)
    W = np.asarray(W, dtype=np.float32)
    b = np.asarray(b, dtype=np.float32)
    C = np.asarray(C, dtype=np.float32)

    nc = _get_graph()
    in_maps = make_in_maps(x, adj, W, b, C)
    res = run_bass_kernel_spmd(nc, in_maps, core_ids=list(range(NCORES)))
    LAST_RESULT = res

    out = np.empty((B, N, H * ATN), dtype=np.float32)
    for core in range(NCORES):
        bb = core // 2
        hg = core % 2
        out[bb, :, hg * P : (hg + 1) * P] = (
            res.results[core]["out"].astype(np.float32).T
        )
    return out

